# revision 22
# baseline (speedup 1.0000x reference)
"""CNN-LSTM (conv1x1 -> LSTM encoder -> LSTM decoder -> dense) on 8 trn2 cores.

Strategy:
  Phase 1 (sharded over the 168 time steps, 21 per core): compute the
  encoder input gate pre-activations xz_t = x_t @ conv_w @ lstm_k + cb2
  in a (gate,batch)x(feature) layout: [128 = 4 gates x 32 batch, 512].
  Phase AG: one AllGather so every core holds all 168 xz_t tiles.
  Phase 2 (replicated on every core; the recurrence is sequential):
  168 encoder steps + 24 decoder steps. Per step: PSUM preloaded with
  xz_t (ACT copy), 16 col-tiled matmuls accumulate h @ rk on top,
  sigmoid/tanh on partition windows, DVE cell update, PE transpose of h
  back to feature-major for the next step's stationary operand.
  Phase 3: dense projection of the 24 decoder hiddens, writes [32,24,512].
"""
import numpy as np
from contextlib import ExitStack

B = 32
H = 168
N = 512
F = 8
NF = N * F          # 4096
G4 = 4 * N          # 2048
P_STEPS = 24
CORES = 8
HC = H // CORES     # 21
TB = HC * B         # 672

_CACHE = {}


def _build_nc(timing=False, relaxed=False):
    from concourse import bass, bacc, tile, mybir, masks

    F32 = mybir.dt.float32
    AF = mybir.ActivationFunctionType
    nc = bacc.Bacc("TRN2", target_bir_lowering=False, debug=False,
                   num_devices=CORES)

    xT_d = nc.declare_dram_parameter("xT", [NF, TB], F32, isOutput=False)
    cw_d = nc.declare_dram_parameter("conv_w", [NF, N], F32, isOutput=False)
    kp_d = nc.declare_dram_parameter("k_p", [N, G4], F32, isOutput=False)
    rkp_d = nc.declare_dram_parameter("rk_p", [N, G4], F32, isOutput=False)
    wdp_d = nc.declare_dram_parameter("wdec_p", [N, G4], F32, isOutput=False)
    cb2_d = nc.declare_dram_parameter("cb2", [4, N], F32, isOutput=False)
    bdec_d = nc.declare_dram_parameter("bdec", [4, N], F32, isOutput=False)
    dw_d = nc.declare_dram_parameter("dense_w", [N, N], F32, isOutput=False)
    dbb_d = nc.declare_dram_parameter("dense_b_bc", [128, N], F32, isOutput=False)
    out_d = nc.declare_dram_parameter("out", [B, P_STEPS, N], F32, isOutput=True)

    F32R = mybir.dt.float32r
    MDT = F32R if relaxed else F32

    def mdt(ap):
        return ap.bitcast(F32R) if relaxed else ap

    def mm(out, lhsT, rhs, start, stop, q=None):
        tp = None if q is None else (0, 32 * q)
        nc.tensor.matmul(out, lhsT, rhs, start=start, stop=stop,
                         tile_position=tp, skip_group_check=True)

    with tile.TileContext(nc) as tc:
        with ExitStack() as octx:
            # ---- persistent small constants
            cpool = octx.enter_context(tc.tile_pool(name="consts", bufs=1))
            ident = cpool.tile([32, 32], F32)
            masks.make_identity(nc, ident[:])
            identr = cpool.tile([32, 32], MDT)
            nc.vector.tensor_copy(identr[:], ident[:])
            ones_f = cpool.tile([1, B], F32)
            nc.vector.memset(ones_f[:], 1.0)
            ones_s = cpool.tile([1, B], MDT)
            nc.vector.tensor_copy(ones_s[:], ones_f[:])
            cb2_s = cpool.tile([1, G4], MDT)
            nc.sync.dma_start(cb2_s[:, :], mdt(cb2_d[:, :].rearrange("g n -> (g n)")[None, :]))
            bdec_s = cpool.tile([1, G4], MDT)
            nc.sync.dma_start(bdec_s[:, :], mdt(bdec_d[:, :].rearrange("g n -> (g n)")[None, :]))

            # ---- DRAM bounce buffers for the AllGather (xz is batch-major
            # [32, 2048] per step so it can be injected into PSUM via an
            # identity-weight matmul with start=True)
            dram = octx.enter_context(tc.tile_pool(name="dram", bufs=1, space="DRAM"))
            xz_loc = dram.tile([HC * B, G4], F32)
            xz_all = dram.tile([H * B, G4], F32,
                               addr_space=("Local" if timing else "Shared"))

            # =============== Phase 1A: xrT = conv_w.T @ xT  ===============
            with ExitStack() as p1:
                sbA = p1.enter_context(tc.tile_pool(name="p1sb", bufs=1))
                xrT_s = sbA.tile([128, 4, TB], MDT)
                pieces = [(s, min(s + 512, TB)) for s in range(0, TB, 512)]
                with ExitStack() as p1a:
                    wpool = p1a.enter_context(tc.tile_pool(name="cwp", bufs=4))
                    xpool = p1a.enter_context(tc.tile_pool(name="xtp", bufs=4))
                    psA = p1a.enter_context(tc.tile_pool(name="psA", bufs=1, space="PSUM"))
                    pAB = [[psA.tile([128, e - s], F32, name=f"pA{m}_{j}")
                            for j, (s, e) in enumerate(pieces)] for m in range(4)]
                    KCH = NF // 128  # 32
                    hb = TB // 2
                    for k in range(KCH):
                        cw_t = wpool.tile([128, N], MDT)
                        nc.sync.dma_start(cw_t[:, :], mdt(cw_d[128 * k:128 * (k + 1), :]))
                        xt_t = xpool.tile([128, TB], MDT)
                        nc.sync.dma_start(xt_t[:, :], mdt(xT_d[128 * k:128 * (k + 1), :]))
                        for m in range(4):
                            for j, (s, e) in enumerate(pieces):
                                mm(pAB[m][j][:, :], cw_t[:, 128 * m:128 * (m + 1)],
                                   xt_t[:, s:e],
                                   start=(k == 0), stop=(k == KCH - 1))
                    for m in range(4):
                        for j, (s, e) in enumerate(pieces):
                            nc.vector.tensor_copy(xrT_s[:, m, s:e], pAB[m][j][:, :])

                # =========== Phase 1B: xz_t = xrT_t.T @ k_p + cb2 ===========
                with ExitStack() as p1b:
                    sbB = p1b.enter_context(tc.tile_pool(name="p1bsb", bufs=1))
                    k_s = sbB.tile([128, 4, G4], MDT)
                    for kk in range(4):
                        nc.sync.dma_start(k_s[:, kk, :], mdt(kp_d[128 * kk:128 * (kk + 1), :]))
                    psB = p1b.enter_context(tc.tile_pool(name="psB", bufs=2, space="PSUM"))
                    evac = p1b.enter_context(tc.tile_pool(name="evac", bufs=3))
                    for t in range(HC):
                        z_ps = psB.tile([128, N], F32)  # (gate,batch)-major
                        for q in range(4):
                            mm(z_ps[32 * q:32 * (q + 1), :], ones_s[:, :],
                               cb2_s[:, N * q:N * (q + 1)], start=True, stop=False, q=q)
                        for kk in range(4):
                            for q in range(4):
                                mm(z_ps[32 * q:32 * (q + 1), :],
                                   xrT_s[:, kk, 32 * t:32 * (t + 1)],
                                   k_s[:, kk, N * q:N * (q + 1)],
                                   start=False, stop=(kk == 3), q=q)
                        # relocate to batch-major [32, 4, 512] (ACT may cross
                        # partition windows; DVE may not)
                        xz_sb = evac.tile([B, 4, N], MDT)
                        for q in range(4):
                            nc.scalar.activation(xz_sb[:, q, :],
                                                 z_ps[32 * q:32 * (q + 1), :], AF.Copy)
                        nc.sync.dma_start(mdt(xz_loc[B * t:B * (t + 1), :]),
                                          xz_sb[:, :, :].rearrange("b q n -> b (q n)"))

            # ======================== AllGather ========================
            if timing:
                # TimelineSim can't model collectives: stand in 8 local DRAM
                # copies with the same data volume as the AllGather.
                for rr in range(CORES):
                    nc.sync.dma_start(
                        xz_all[HC * B * rr:HC * B * (rr + 1), :], xz_loc[:, :])
            else:
                nc.gpsimd.collective_compute(
                    "AllGather", bass.mybir.AluOpType.bypass,
                    replica_groups=[list(range(CORES))],
                    ins=[xz_loc.opt()], outs=[xz_all.opt()],
                )

            # ================= Phase 2: the recurrence =================
            sb2 = octx.enter_context(tc.tile_pool(name="p2sb", bufs=1))
            rk_s = sb2.tile([128, 4, G4], MDT)
            wd_s = sb2.tile([128, 4, G4], MDT)
            for kk in range(4):
                nc.sync.dma_start(rk_s[:, kk, :], mdt(rkp_d[128 * kk:128 * (kk + 1), :]))
                nc.sync.dma_start(wd_s[:, kk, :], mdt(wdp_d[128 * kk:128 * (kk + 1), :]))

            hT_f = sb2.tile([128, 128], F32)
            nc.vector.memset(hT_f[:], 0.0)
            hT_s = sb2.tile([128, 128], MDT)
            nc.vector.tensor_copy(hT_s[:], hT_f[:])
            c_s = sb2.tile([B, N], F32)
            nc.vector.memset(c_s[:], 0.0)
            si_s = sb2.tile([B, N], F32)       # sigmoid(i)
            sf_s = sb2.tile([B, N], F32)       # sigmoid(f)
            so_s = sb2.tile([B, N], F32)       # sigmoid(o)
            tg_s = sb2.tile([B, N], F32)       # tanh(g)
            t1_s = sb2.tile([B, N], F32)
            t2_s = sb2.tile([B, N], F32)
            tc_s = sb2.tile([B, N], F32)
            h_s = sb2.tile([B, N], F32)
            predsT_s = sb2.tile([128, 4, P_STEPS * B], MDT)

            xzp = octx.enter_context(tc.tile_pool(name="xzp", bufs=6))
            ps2 = octx.enter_context(tc.tile_pool(name="ps2", bufs=3, space="PSUM"))
            trp = octx.enter_context(tc.tile_pool(name="trp", bufs=2, space="PSUM"))

            def step(t, enc):
                """One LSTM step, batch-major [32, 2048]: all elementwise ops
                stay on partitions 0:32; gates live in the free dimension."""
                z_ps = ps2.tile([128, N], F32, name="z_ps")  # (gate,batch)-major
                if enc:
                    xz_t = xzp.tile([B, 4, N], MDT, name="xz_t")
                    nc.sync.dma_start(xz_t[:, :, :].rearrange("b q n -> b (q n)"),
                                      mdt(xz_all[B * t:B * (t + 1), :]))
                    # inject xz into PSUM via identity-weight matmuls, one per
                    # column group (concurrent on the PE array)
                    for q in range(4):
                        mm(z_ps[32 * q:32 * (q + 1), :], identr[:, :],
                           xz_t[:, q, :], start=True, stop=False, q=q)
                    w_s = rk_s
                else:
                    for q in range(4):
                        mm(z_ps[32 * q:32 * (q + 1), :], ones_s[:, :],
                           bdec_s[:, N * q:N * (q + 1)], start=True, stop=False, q=q)
                    w_s = wd_s
                if enc or t == 0:
                    lhs = lambda kk: hT_s[:, 32 * kk:32 * (kk + 1)]
                else:
                    lhs = lambda kk: predsT_s[:, kk, B * (t - 1):B * t]
                for kk in range(4):
                    for q in range(4):
                        mm(z_ps[32 * q:32 * (q + 1), :], lhs(kk),
                           w_s[:, kk, N * q:N * (q + 1)],
                           start=False, stop=(kk == 3), q=q)
                # activations, relocating each gate to partitions 0:32 (ACT
                # cross-window reads are legal; DVE ops below stay aligned).
                # slots: [i | f | o | g]
                nc.scalar.activation(sf_s[:, :], z_ps[32:64, :], AF.Sigmoid)
                nc.scalar.activation(tg_s[:, :], z_ps[96:128, :], AF.Tanh)
                nc.scalar.activation(si_s[:, :], z_ps[0:32, :], AF.Sigmoid)
                nc.scalar.activation(so_s[:, :], z_ps[64:96, :], AF.Sigmoid)
                # c = f*c + i*g ; h = o * tanh(c)
                nc.vector.tensor_mul(t2_s[:, :], sf_s[:, :], c_s[:, :])
                nc.vector.tensor_mul(t1_s[:, :], si_s[:, :], tg_s[:, :])
                nc.vector.tensor_add(c_s[:, :], t1_s[:, :], t2_s[:, :])
                nc.scalar.activation(tc_s[:, :], c_s[:, :], AF.Tanh)
                nc.vector.tensor_mul(h_s[:, :], so_s[:, :], tc_s[:, :])
                # transpose h [32,512] -> feature-major [128, 4*32]
                tr = trp.tile([128, 128], F32, name="tr")
                for kk in range(4):
                    nc.tensor.transpose(tr[:, 32 * kk:32 * (kk + 1)],
                                        h_s[:, 128 * kk:128 * (kk + 1)], ident[:, :])
                    dst = (hT_s[:, 32 * kk:32 * (kk + 1)] if enc
                           else predsT_s[:, kk, B * t:B * (t + 1)])
                    nc.vector.tensor_copy(dst, tr[:, 32 * kk:32 * (kk + 1)])

            for t in range(H):
                step(t, enc=True)
            for t in range(P_STEPS):
                step(t, enc=False)

            # ================= Phase 3: dense head =================
            dw_s = sb2.tile([128, 4, N], MDT)
            for kk in range(4):
                nc.sync.dma_start(dw_s[:, kk, :], mdt(dw_d[128 * kk:128 * (kk + 1), :]))
            db_s = sb2.tile([128, N], F32)
            nc.sync.dma_start(db_s[:, :], dbb_d[:, :])
            ps3 = octx.enter_context(tc.tile_pool(name="ps3", bufs=2, space="PSUM"))
            op3 = octx.enter_context(tc.tile_pool(name="op3", bufs=2))
            MT = (P_STEPS * B) // 128  # 6
            out_r = out_d[:, :, :].rearrange("b t n -> t b n")
            for m in range(MT):
                dps = ps3.tile([128, N], F32, name="dps")
                for kk in range(4):
                    mm(dps[:, :], predsT_s[:, kk, 128 * m:128 * (m + 1)],
                       dw_s[:, kk, :], start=(kk == 0), stop=(kk == 3))
                o_sb = op3.tile([128, N], F32, name="o_sb")
                nc.vector.tensor_add(o_sb[:, :], dps[:, :], db_s[:, :])
                for j in range(4):
                    nc.sync.dma_start(out_r[4 * m + j, :, :], o_sb[B * j:B * (j + 1), :])
    return nc


def _host_prep(x, conv_w, conv_b, lstm_k, lstm_rk, lstm_b, dense_w, dense_b):
    x = np.asarray(x, dtype=np.float32)
    conv_w = np.asarray(conv_w, dtype=np.float32)
    conv_b = np.asarray(conv_b, dtype=np.float32)
    lstm_k = np.asarray(lstm_k, dtype=np.float32)
    lstm_rk = np.asarray(lstm_rk, dtype=np.float32)
    lstm_b = np.asarray(lstm_b, dtype=np.float32)
    dense_w = np.asarray(dense_w, dtype=np.float32)
    dense_b = np.asarray(dense_b, dtype=np.float32)

    # gate slot order [i | f | o | g] (keras order in weights is i,f,g,o)
    perm = np.r_[0:N, N:2 * N, 3 * N:4 * N, 2 * N:3 * N]
    k_p = np.ascontiguousarray(lstm_k[:, perm])
    rk_p = np.ascontiguousarray(lstm_rk[:, perm])
    wdec_p = np.ascontiguousarray((lstm_k + lstm_rk)[:, perm])
    cb2 = (conv_b @ lstm_k + lstm_b)[perm].reshape(4, N)
    bdec = lstm_b[perm].reshape(4, N)
    db_bc = np.ascontiguousarray(np.broadcast_to(dense_b, (128, N)))

    shared = {
        "conv_w": conv_w, "k_p": k_p, "rk_p": rk_p, "wdec_p": wdec_p,
        "cb2": np.ascontiguousarray(cb2), "bdec": np.ascontiguousarray(bdec),
        "dense_w": dense_w, "dense_b_bc": db_bc,
    }
    in_maps = []
    for c in range(CORES):
        xs = x[:, HC * c:HC * (c + 1)].reshape(B, HC, NF)
        xT = np.ascontiguousarray(xs.transpose(2, 1, 0).reshape(NF, TB))
        in_maps.append({"xT": xT, **shared})
    return in_maps


class _Runner:
    """Builds the SPMD PJRT executable once; reusable across calls.

    Mirrors concourse.bass2jax.run_bass_via_pjrt's multi-core branch but
    keeps the jitted callable so repeat executions don't recompile.
    """

    def __init__(self, nc):
        import jax
        from jax.sharding import Mesh, PartitionSpec, NamedSharding
        from jax.experimental.shard_map import shard_map
        from concourse import bass2jax, mybir

        bass2jax.install_neuronx_cc_hook()
        if not nc.is_finalized():
            nc.finalize()
        self.nc = nc
        partition_name = (nc.partition_id_tensor.name
                          if nc.partition_id_tensor else None)
        in_names, out_names, out_avals = [], [], []
        for alloc in nc.m.functions[0].allocations:
            if not isinstance(alloc, mybir.MemoryLocationSet):
                continue
            name = alloc.memorylocations[0].name
            if alloc.kind == "ExternalInput":
                if name != partition_name:
                    in_names.append(name)
            elif alloc.kind == "ExternalOutput":
                shape = tuple(alloc.tensor_shape)
                dtype = mybir.dt.np(alloc.dtype)
                out_names.append(name)
                out_avals.append(jax.core.ShapedArray(shape, dtype))
        self.n_params = len(in_names)
        self.in_names = list(in_names)
        self.out_names = out_names
        self.out_avals = out_avals
        all_names = in_names + out_names
        if partition_name is not None:
            all_names.append(partition_name)

        def _body(*args):
            operands = list(args)
            if partition_name is not None:
                operands.append(bass2jax.partition_id_tensor())
            outs = bass2jax._bass_exec_p.bind(
                *operands,
                out_avals=tuple(out_avals),
                in_names=tuple(all_names),
                out_names=tuple(out_names),
                lowering_input_output_aliases=(),
                sim_require_finite=True,
                sim_require_nnan=True,
                nc=nc,
            )
            return tuple(outs)

        devices = jax.devices()[:CORES]
        self.mesh = Mesh(np.asarray(devices), ("core",))
        self.sharding = NamedSharding(self.mesh, PartitionSpec("core"))
        n_io = self.n_params + len(out_names)
        self.fn = jax.jit(
            shard_map(_body, mesh=self.mesh,
                      in_specs=(PartitionSpec("core"),) * n_io,
                      out_specs=(PartitionSpec("core"),) * len(out_names),
                      check_rep=False),
            keep_unused=True)
        self._jax = jax
        self._zeros = None

    def put_inputs(self, in_maps):
        jax = self._jax
        concat = [np.concatenate([np.asarray(m[n]) for m in in_maps], axis=0)
                  for n in self.in_names]
        return [jax.device_put(a, self.sharding) for a in concat]

    def fresh_zeros(self):
        jax = self._jax
        if self._zeros is None:
            self._zeros = [jax.device_put(
                np.zeros((CORES * av.shape[0], *av.shape[1:]), av.dtype),
                self.sharding) for av in self.out_avals]
        return self._zeros

    def run(self, dev_in):
        outs = self.fn(*dev_in, *self.fresh_zeros())
        return [np.asarray(o) for o in outs]


def _get_runner():
    if "runner" not in _CACHE:
        _CACHE["runner"] = _Runner(_build_nc())
    return _CACHE["runner"]


def kernel(x, conv_w, conv_b, lstm_k, lstm_rk, lstm_b, dense_w, dense_b, P):
    in_maps = _host_prep(x, conv_w, conv_b, lstm_k, lstm_rk,
                         lstm_b, dense_w, dense_b)
    last_err = None
    for attempt in range(3):
        try:
            r = _get_runner()
            outs = r.run(r.put_inputs(in_maps))
            full = outs[r.out_names.index("out")]
            return (np.asarray(full).reshape(CORES, B, P_STEPS, N)[0]
                    .astype(np.float32))
        except Exception as e:  # transient NRT wedges recover on retry
            last_err = e
            _CACHE.clear()
            import time as _time
            _time.sleep(5)
    raise last_err


# revision 23
# speedup vs baseline: 1.3160x; 1.3160x over previous
"""CNN-LSTM (conv1x1 -> LSTM encoder -> LSTM decoder -> dense) on 8 trn2 cores.

Strategy:
  Phase 1 (sharded over the 168 time steps, 21 per core): compute the
  encoder input gate pre-activations xz_t = x_t @ conv_w @ lstm_k + cb2
  in a (gate,batch)x(feature) layout: [128 = 4 gates x 32 batch, 512].
  Phase AG: one AllGather so every core holds all 168 xz_t tiles.
  Phase 2 (replicated on every core; the recurrence is sequential):
  168 encoder steps + 24 decoder steps. Per step: PSUM preloaded with
  xz_t (ACT copy), 16 col-tiled matmuls accumulate h @ rk on top,
  sigmoid/tanh on partition windows, DVE cell update, PE transpose of h
  back to feature-major for the next step's stationary operand.
  Phase 3: dense projection of the 24 decoder hiddens, writes [32,24,512].
"""
import numpy as np
from contextlib import ExitStack

B = 32
H = 168
N = 512
F = 8
NF = N * F          # 4096
G4 = 4 * N          # 2048
P_STEPS = 24
CORES = 8
HC = H // CORES     # 21
TB = HC * B         # 672

_CACHE = {}


def _build_nc(timing=False, relaxed=False):
    from concourse import bass, bacc, tile, mybir, masks

    F32 = mybir.dt.float32
    AF = mybir.ActivationFunctionType
    nc = bacc.Bacc("TRN2", target_bir_lowering=False, debug=False,
                   num_devices=CORES)

    xT_d = nc.declare_dram_parameter("xT", [NF, TB], F32, isOutput=False)
    cw_d = nc.declare_dram_parameter("conv_w", [NF, N], F32, isOutput=False)
    kp_d = nc.declare_dram_parameter("k_p", [N, G4], F32, isOutput=False)
    rkp_d = nc.declare_dram_parameter("rk_p", [N, G4], F32, isOutput=False)
    wdp_d = nc.declare_dram_parameter("wdec_p", [N, G4], F32, isOutput=False)
    cb2_d = nc.declare_dram_parameter("cb2", [4, N], F32, isOutput=False)
    bdec_d = nc.declare_dram_parameter("bdec", [4, N], F32, isOutput=False)
    dw_d = nc.declare_dram_parameter("dense_w", [N, N], F32, isOutput=False)
    dbb_d = nc.declare_dram_parameter("dense_b_bc", [128, N], F32, isOutput=False)
    out_d = nc.declare_dram_parameter("out", [B, P_STEPS, N], F32, isOutput=True)

    F32R = mybir.dt.float32r
    MDT = F32R if relaxed else F32

    def mdt(ap):
        return ap.bitcast(F32R) if relaxed else ap

    def mm(out, lhsT, rhs, start, stop, q=None):
        tp = None if q is None else (0, 32 * q)
        nc.tensor.matmul(out, lhsT, rhs, start=start, stop=stop,
                         tile_position=tp, skip_group_check=True)

    with tile.TileContext(nc) as tc:
        with ExitStack() as octx:
            # ---- persistent small constants
            cpool = octx.enter_context(tc.tile_pool(name="consts", bufs=1))
            ident = cpool.tile([32, 32], F32)
            masks.make_identity(nc, ident[:])
            identr = cpool.tile([32, 32], MDT)
            nc.vector.tensor_copy(identr[:], ident[:])
            ones_f = cpool.tile([1, B], F32)
            nc.vector.memset(ones_f[:], 1.0)
            ones_s = cpool.tile([1, B], MDT)
            nc.vector.tensor_copy(ones_s[:], ones_f[:])
            cb2_s = cpool.tile([1, G4], MDT)
            nc.sync.dma_start(cb2_s[:, :], mdt(cb2_d[:, :].rearrange("g n -> (g n)")[None, :]))
            bdec_s = cpool.tile([1, G4], MDT)
            nc.sync.dma_start(bdec_s[:, :], mdt(bdec_d[:, :].rearrange("g n -> (g n)")[None, :]))

            # ---- DRAM bounce buffers for the AllGather (xz is batch-major
            # [32, 2048] per step so it can be injected into PSUM via an
            # identity-weight matmul with start=True)
            dram = octx.enter_context(tc.tile_pool(name="dram", bufs=1, space="DRAM"))
            xz_loc = dram.tile([HC * B, G4], F32)
            xz_all = dram.tile([H * B, G4], F32,
                               addr_space=("Local" if timing else "Shared"))

            # =============== Phase 1A: xrT = conv_w.T @ xT  ===============
            with ExitStack() as p1:
                sbA = p1.enter_context(tc.tile_pool(name="p1sb", bufs=1))
                xrT_s = sbA.tile([128, 4, TB], MDT)
                pieces = [(s, min(s + 512, TB)) for s in range(0, TB, 512)]
                with ExitStack() as p1a:
                    wpool = p1a.enter_context(tc.tile_pool(name="cwp", bufs=6))
                    xpool = p1a.enter_context(tc.tile_pool(name="xtp", bufs=6))
                    psA = p1a.enter_context(tc.tile_pool(name="psA", bufs=1, space="PSUM"))
                    pAB = [[psA.tile([128, e - s], F32, name=f"pA{m}_{j}")
                            for j, (s, e) in enumerate(pieces)] for m in range(4)]
                    KCH = NF // 128  # 32
                    hb = TB // 2
                    for k in range(KCH):
                        cw_t = wpool.tile([128, N], MDT)
                        nc.sync.dma_start(cw_t[:, :], mdt(cw_d[128 * k:128 * (k + 1), :]))
                        xt_t = xpool.tile([128, TB], MDT)
                        nc.sync.dma_start(xt_t[:, :], mdt(xT_d[128 * k:128 * (k + 1), :]))
                        for m in range(4):
                            for j, (s, e) in enumerate(pieces):
                                mm(pAB[m][j][:, :], cw_t[:, 128 * m:128 * (m + 1)],
                                   xt_t[:, s:e],
                                   start=(k == 0), stop=(k == KCH - 1))
                    for m in range(4):
                        for j, (s, e) in enumerate(pieces):
                            nc.vector.tensor_copy(xrT_s[:, m, s:e], pAB[m][j][:, :])

                # =========== Phase 1B: xz_t = xrT_t.T @ k_p + cb2 ===========
                with ExitStack() as p1b:
                    sbB = p1b.enter_context(tc.tile_pool(name="p1bsb", bufs=1))
                    k_s = sbB.tile([128, 4, G4], MDT)
                    for kk in range(4):
                        nc.sync.dma_start(k_s[:, kk, :], mdt(kp_d[128 * kk:128 * (kk + 1), :]))
                    psB = p1b.enter_context(tc.tile_pool(name="psB", bufs=2, space="PSUM"))
                    evac = p1b.enter_context(tc.tile_pool(name="evac", bufs=3))
                    for t in range(HC):
                        z_ps = psB.tile([128, N], F32)  # (gate,batch)-major
                        for q in range(4):
                            mm(z_ps[32 * q:32 * (q + 1), :], ones_s[:, :],
                               cb2_s[:, N * q:N * (q + 1)], start=True, stop=False, q=q)
                        for kk in range(4):
                            for q in range(4):
                                mm(z_ps[32 * q:32 * (q + 1), :],
                                   xrT_s[:, kk, 32 * t:32 * (t + 1)],
                                   k_s[:, kk, N * q:N * (q + 1)],
                                   start=False, stop=(kk == 3), q=q)
                        # relocate to batch-major [32, 4, 512] (ACT may cross
                        # partition windows; DVE may not)
                        xz_sb = evac.tile([B, 4, N], MDT)
                        for q in range(4):
                            nc.scalar.activation(xz_sb[:, q, :],
                                                 z_ps[32 * q:32 * (q + 1), :], AF.Copy)
                        nc.sync.dma_start(mdt(xz_loc[B * t:B * (t + 1), :]),
                                          xz_sb[:, :, :].rearrange("b q n -> b (q n)"))

            # ======================== AllGather ========================
            if timing:
                # TimelineSim can't model collectives: stand in 8 local DRAM
                # copies with the same data volume as the AllGather.
                for rr in range(CORES):
                    nc.sync.dma_start(
                        xz_all[HC * B * rr:HC * B * (rr + 1), :], xz_loc[:, :])
            else:
                nc.gpsimd.collective_compute(
                    "AllGather", bass.mybir.AluOpType.bypass,
                    replica_groups=[list(range(CORES))],
                    ins=[xz_loc.opt()], outs=[xz_all.opt()],
                )

            # ================= Phase 2: the recurrence =================
            sb2 = octx.enter_context(tc.tile_pool(name="p2sb", bufs=1))
            rk_s = sb2.tile([128, 4, G4], MDT)
            wd_s = sb2.tile([128, 4, G4], MDT)
            for kk in range(4):
                nc.sync.dma_start(rk_s[:, kk, :], mdt(rkp_d[128 * kk:128 * (kk + 1), :]))
                nc.sync.dma_start(wd_s[:, kk, :], mdt(wdp_d[128 * kk:128 * (kk + 1), :]))

            hT_f = sb2.tile([128, 128], F32)
            nc.vector.memset(hT_f[:], 0.0)
            hT_s = sb2.tile([128, 128], MDT)
            nc.vector.tensor_copy(hT_s[:], hT_f[:])
            c_s = sb2.tile([B, N], F32)
            nc.vector.memset(c_s[:], 0.0)
            si_s = sb2.tile([B, N], F32)       # sigmoid(i)
            sf_s = sb2.tile([B, N], F32)       # sigmoid(f)
            so_s = sb2.tile([B, N], F32)       # sigmoid(o)
            tg_s = sb2.tile([B, N], F32)       # tanh(g)
            t1_s = sb2.tile([B, N], F32)
            t2_s = sb2.tile([B, N], F32)
            tc_s = sb2.tile([B, N], F32)
            h_s = sb2.tile([B, N], F32)
            predsT_s = sb2.tile([128, 4, P_STEPS * B], MDT)

            xzp = octx.enter_context(tc.tile_pool(name="xzp", bufs=8))
            ps2 = octx.enter_context(tc.tile_pool(name="ps2", bufs=3, space="PSUM"))
            trp = octx.enter_context(tc.tile_pool(name="trp", bufs=2, space="PSUM"))

            def step(t, enc):
                """One LSTM step, batch-major [32, 2048]: all elementwise ops
                stay on partitions 0:32; gates live in the free dimension."""
                z_ps = ps2.tile([128, N], F32, name="z_ps")  # (gate,batch)-major
                if enc:
                    xz_t = xzp.tile([B, 4, N], MDT, name="xz_t")
                    nc.sync.dma_start(xz_t[:, :, :].rearrange("b q n -> b (q n)"),
                                      mdt(xz_all[B * t:B * (t + 1), :]))
                    # inject xz into PSUM via identity-weight matmuls, one per
                    # column group (concurrent on the PE array)
                    for q in range(4):
                        mm(z_ps[32 * q:32 * (q + 1), :], identr[:, :],
                           xz_t[:, q, :], start=True, stop=False, q=q)
                    w_s = rk_s
                else:
                    for q in range(4):
                        mm(z_ps[32 * q:32 * (q + 1), :], ones_s[:, :],
                           bdec_s[:, N * q:N * (q + 1)], start=True, stop=False, q=q)
                    w_s = wd_s
                if enc or t == 0:
                    lhs = lambda kk: hT_s[:, 32 * kk:32 * (kk + 1)]
                else:
                    lhs = lambda kk: predsT_s[:, kk, B * (t - 1):B * t]
                for kk in range(4):
                    for q in range(4):
                        mm(z_ps[32 * q:32 * (q + 1), :], lhs(kk),
                           w_s[:, kk, N * q:N * (q + 1)],
                           start=False, stop=(kk == 3), q=q)
                # activations, relocating each gate to partitions 0:32 (ACT
                # cross-window reads are legal; DVE ops below stay aligned).
                # slots: [i | f | o | g]
                nc.scalar.activation(sf_s[:, :], z_ps[32:64, :], AF.Sigmoid)
                nc.scalar.activation(tg_s[:, :], z_ps[96:128, :], AF.Tanh)
                nc.scalar.activation(si_s[:, :], z_ps[0:32, :], AF.Sigmoid)
                nc.scalar.activation(so_s[:, :], z_ps[64:96, :], AF.Sigmoid)
                # c = f*c + i*g ; h = o * tanh(c)
                nc.vector.tensor_mul(t2_s[:, :], sf_s[:, :], c_s[:, :])
                nc.vector.tensor_mul(t1_s[:, :], si_s[:, :], tg_s[:, :])
                nc.vector.tensor_add(c_s[:, :], t1_s[:, :], t2_s[:, :])
                nc.scalar.activation(tc_s[:, :], c_s[:, :], AF.Tanh)
                nc.vector.tensor_mul(h_s[:, :], so_s[:, :], tc_s[:, :])
                # transpose h [32,512] -> feature-major [128, 4*32]
                tr = trp.tile([128, 128], F32, name="tr")
                for kk in range(4):
                    nc.tensor.transpose(tr[:, 32 * kk:32 * (kk + 1)],
                                        h_s[:, 128 * kk:128 * (kk + 1)], ident[:, :])
                    dst = (hT_s[:, 32 * kk:32 * (kk + 1)] if enc
                           else predsT_s[:, kk, B * t:B * (t + 1)])
                    nc.vector.tensor_copy(dst, tr[:, 32 * kk:32 * (kk + 1)])

            for t in range(H):
                step(t, enc=True)
            for t in range(P_STEPS):
                step(t, enc=False)

            # ================= Phase 3: dense head =================
            dw_s = sb2.tile([128, 4, N], MDT)
            for kk in range(4):
                nc.sync.dma_start(dw_s[:, kk, :], mdt(dw_d[128 * kk:128 * (kk + 1), :]))
            db_s = sb2.tile([128, N], F32)
            nc.sync.dma_start(db_s[:, :], dbb_d[:, :])
            ps3 = octx.enter_context(tc.tile_pool(name="ps3", bufs=2, space="PSUM"))
            op3 = octx.enter_context(tc.tile_pool(name="op3", bufs=2))
            MT = (P_STEPS * B) // 128  # 6
            out_r = out_d[:, :, :].rearrange("b t n -> t b n")
            for m in range(MT):
                dps = ps3.tile([128, N], F32, name="dps")
                for kk in range(4):
                    mm(dps[:, :], predsT_s[:, kk, 128 * m:128 * (m + 1)],
                       dw_s[:, kk, :], start=(kk == 0), stop=(kk == 3))
                o_sb = op3.tile([128, N], F32, name="o_sb")
                nc.vector.tensor_add(o_sb[:, :], dps[:, :], db_s[:, :])
                for j in range(4):
                    nc.sync.dma_start(out_r[4 * m + j, :, :], o_sb[B * j:B * (j + 1), :])
    return nc


def _host_prep(x, conv_w, conv_b, lstm_k, lstm_rk, lstm_b, dense_w, dense_b):
    x = np.asarray(x, dtype=np.float32)
    conv_w = np.asarray(conv_w, dtype=np.float32)
    conv_b = np.asarray(conv_b, dtype=np.float32)
    lstm_k = np.asarray(lstm_k, dtype=np.float32)
    lstm_rk = np.asarray(lstm_rk, dtype=np.float32)
    lstm_b = np.asarray(lstm_b, dtype=np.float32)
    dense_w = np.asarray(dense_w, dtype=np.float32)
    dense_b = np.asarray(dense_b, dtype=np.float32)

    # gate slot order [i | f | o | g] (keras order in weights is i,f,g,o)
    perm = np.r_[0:N, N:2 * N, 3 * N:4 * N, 2 * N:3 * N]
    k_p = np.ascontiguousarray(lstm_k[:, perm])
    rk_p = np.ascontiguousarray(lstm_rk[:, perm])
    wdec_p = np.ascontiguousarray((lstm_k + lstm_rk)[:, perm])
    cb2 = (conv_b @ lstm_k + lstm_b)[perm].reshape(4, N)
    bdec = lstm_b[perm].reshape(4, N)
    db_bc = np.ascontiguousarray(np.broadcast_to(dense_b, (128, N)))

    shared = {
        "conv_w": conv_w, "k_p": k_p, "rk_p": rk_p, "wdec_p": wdec_p,
        "cb2": np.ascontiguousarray(cb2), "bdec": np.ascontiguousarray(bdec),
        "dense_w": dense_w, "dense_b_bc": db_bc,
    }
    in_maps = []
    for c in range(CORES):
        xs = x[:, HC * c:HC * (c + 1)].reshape(B, HC, NF)
        xT = np.ascontiguousarray(xs.transpose(2, 1, 0).reshape(NF, TB))
        in_maps.append({"xT": xT, **shared})
    return in_maps


class _Runner:
    """Builds the SPMD PJRT executable once; reusable across calls.

    Mirrors concourse.bass2jax.run_bass_via_pjrt's multi-core branch but
    keeps the jitted callable so repeat executions don't recompile.
    """

    def __init__(self, nc):
        import jax
        from jax.sharding import Mesh, PartitionSpec, NamedSharding
        from jax.experimental.shard_map import shard_map
        from concourse import bass2jax, mybir

        bass2jax.install_neuronx_cc_hook()
        if not nc.is_finalized():
            nc.finalize()
        self.nc = nc
        partition_name = (nc.partition_id_tensor.name
                          if nc.partition_id_tensor else None)
        in_names, out_names, out_avals = [], [], []
        for alloc in nc.m.functions[0].allocations:
            if not isinstance(alloc, mybir.MemoryLocationSet):
                continue
            name = alloc.memorylocations[0].name
            if alloc.kind == "ExternalInput":
                if name != partition_name:
                    in_names.append(name)
            elif alloc.kind == "ExternalOutput":
                shape = tuple(alloc.tensor_shape)
                dtype = mybir.dt.np(alloc.dtype)
                out_names.append(name)
                out_avals.append(jax.core.ShapedArray(shape, dtype))
        self.n_params = len(in_names)
        self.in_names = list(in_names)
        self.out_names = out_names
        self.out_avals = out_avals
        all_names = in_names + out_names
        if partition_name is not None:
            all_names.append(partition_name)

        def _body(*args):
            operands = list(args)
            if partition_name is not None:
                operands.append(bass2jax.partition_id_tensor())
            outs = bass2jax._bass_exec_p.bind(
                *operands,
                out_avals=tuple(out_avals),
                in_names=tuple(all_names),
                out_names=tuple(out_names),
                lowering_input_output_aliases=(),
                sim_require_finite=True,
                sim_require_nnan=True,
                nc=nc,
            )
            return tuple(outs)

        devices = jax.devices()[:CORES]
        self.mesh = Mesh(np.asarray(devices), ("core",))
        self.sharding = NamedSharding(self.mesh, PartitionSpec("core"))
        n_io = self.n_params + len(out_names)
        self.fn = jax.jit(
            shard_map(_body, mesh=self.mesh,
                      in_specs=(PartitionSpec("core"),) * n_io,
                      out_specs=(PartitionSpec("core"),) * len(out_names),
                      check_rep=False),
            keep_unused=True)
        self._jax = jax
        self._zeros = None

    def put_inputs(self, in_maps):
        jax = self._jax
        concat = [np.concatenate([np.asarray(m[n]) for m in in_maps], axis=0)
                  for n in self.in_names]
        return [jax.device_put(a, self.sharding) for a in concat]

    def fresh_zeros(self):
        jax = self._jax
        if self._zeros is None:
            self._zeros = [jax.device_put(
                np.zeros((CORES * av.shape[0], *av.shape[1:]), av.dtype),
                self.sharding) for av in self.out_avals]
        return self._zeros

    def run(self, dev_in):
        outs = self.fn(*dev_in, *self.fresh_zeros())
        return [np.asarray(o) for o in outs]


def _get_runner():
    if "runner" not in _CACHE:
        _CACHE["runner"] = _Runner(_build_nc())
    return _CACHE["runner"]


def kernel(x, conv_w, conv_b, lstm_k, lstm_rk, lstm_b, dense_w, dense_b, P):
    in_maps = _host_prep(x, conv_w, conv_b, lstm_k, lstm_rk,
                         lstm_b, dense_w, dense_b)
    last_err = None
    for attempt in range(3):
        try:
            r = _get_runner()
            outs = r.run(r.put_inputs(in_maps))
            full = outs[r.out_names.index("out")]
            return (np.asarray(full).reshape(CORES, B, P_STEPS, N)[0]
                    .astype(np.float32))
        except Exception as e:  # transient NRT wedges recover on retry
            last_err = e
            _CACHE.clear()
            import time as _time
            _time.sleep(5)
    raise last_err


# revision 25
# speedup vs baseline: 1.7850x; 1.3564x over previous
"""CNN-LSTM (conv1x1 -> LSTM encoder -> LSTM decoder -> dense) on 8 trn2 cores.

Strategy:
  Phase 1 (sharded over the 168 time steps, 21 per core): compute the
  encoder input gate pre-activations xz_t = x_t @ conv_w @ lstm_k + cb2
  in a (gate,batch)x(feature) layout: [128 = 4 gates x 32 batch, 512].
  Phase AG: one AllGather so every core holds all 168 xz_t tiles.
  Phase 2 (replicated on every core; the recurrence is sequential):
  168 encoder steps + 24 decoder steps. Per step: PSUM preloaded with
  xz_t (ACT copy), 16 col-tiled matmuls accumulate h @ rk on top,
  sigmoid/tanh on partition windows, DVE cell update, PE transpose of h
  back to feature-major for the next step's stationary operand.
  Phase 3: dense projection of the 24 decoder hiddens, writes [32,24,512].
"""
import numpy as np
from contextlib import ExitStack

B = 32
H = 168
N = 512
F = 8
NF = N * F          # 4096
G4 = 4 * N          # 2048
P_STEPS = 24
CORES = 8
HC = H // CORES     # 21
TB = HC * B         # 672

_CACHE = {}


def _build_nc(timing=False, relaxed=False):
    from concourse import bass, bacc, tile, mybir, masks

    F32 = mybir.dt.float32
    AF = mybir.ActivationFunctionType
    nc = bacc.Bacc("TRN2", target_bir_lowering=False, debug=False,
                   num_devices=CORES)

    xT_d = nc.declare_dram_parameter("xT", [NF, TB], F32, isOutput=False)
    cw_d = nc.declare_dram_parameter("conv_w", [NF, N], F32, isOutput=False)
    kp_d = nc.declare_dram_parameter("k_p", [N, G4], F32, isOutput=False)
    rkp_d = nc.declare_dram_parameter("rk_p", [N, G4], F32, isOutput=False)
    wdp_d = nc.declare_dram_parameter("wdec_p", [N, G4], F32, isOutput=False)
    cb2_d = nc.declare_dram_parameter("cb2", [4, N], F32, isOutput=False)
    bdec_d = nc.declare_dram_parameter("bdec", [4, N], F32, isOutput=False)
    dw_d = nc.declare_dram_parameter("dense_w", [N, N], F32, isOutput=False)
    dbb_d = nc.declare_dram_parameter("dense_b_bc", [128, N], F32, isOutput=False)
    out_d = nc.declare_dram_parameter("out", [B, P_STEPS, N], F32, isOutput=True)

    F32R = mybir.dt.float32r
    MDT = F32R if relaxed else F32

    def mdt(ap):
        return ap.bitcast(F32R) if relaxed else ap

    def mm(out, lhsT, rhs, start, stop, q=None):
        tp = None if q is None else (0, 32 * q)
        nc.tensor.matmul(out, lhsT, rhs, start=start, stop=stop,
                         tile_position=tp, skip_group_check=True)

    with tile.TileContext(nc) as tc:
        with ExitStack() as octx:
            # ---- persistent small constants
            cpool = octx.enter_context(tc.tile_pool(name="consts", bufs=1))
            ident = cpool.tile([32, 32], F32)
            masks.make_identity(nc, ident[:])
            identr = cpool.tile([32, 32], MDT)
            nc.vector.tensor_copy(identr[:], ident[:])
            ones_f = cpool.tile([1, B], F32)
            nc.vector.memset(ones_f[:], 1.0)
            ones_s = cpool.tile([1, B], MDT)
            nc.vector.tensor_copy(ones_s[:], ones_f[:])
            cb2_s = cpool.tile([1, G4], MDT)
            nc.sync.dma_start(cb2_s[:, :], mdt(cb2_d[:, :].rearrange("g n -> (g n)")[None, :]))
            bdec_s = cpool.tile([1, G4], MDT)
            nc.sync.dma_start(bdec_s[:, :], mdt(bdec_d[:, :].rearrange("g n -> (g n)")[None, :]))

            # ---- DRAM bounce buffers for the AllGather (xz is batch-major
            # [32, 2048] per step so it can be injected into PSUM via an
            # identity-weight matmul with start=True)
            dram = octx.enter_context(tc.tile_pool(name="dram", bufs=1, space="DRAM"))
            xz_loc = dram.tile([HC * B, G4], F32)
            xz_all = dram.tile([H * B, G4], F32,
                               addr_space=("Local" if timing else "Shared"))

            # =============== Phase 1A: xrT = conv_w.T @ xT  ===============
            with ExitStack() as p1:
                sbA = p1.enter_context(tc.tile_pool(name="p1sb", bufs=1))
                xrT_s = sbA.tile([128, 4, TB], MDT)
                pieces = [(s, min(s + 512, TB)) for s in range(0, TB, 512)]
                with ExitStack() as p1a:
                    wpool = p1a.enter_context(tc.tile_pool(name="cwp", bufs=6))
                    xpool = p1a.enter_context(tc.tile_pool(name="xtp", bufs=6))
                    psA = p1a.enter_context(tc.tile_pool(name="psA", bufs=1, space="PSUM"))
                    pAB = [[psA.tile([128, e - s], F32, name=f"pA{m}_{j}")
                            for j, (s, e) in enumerate(pieces)] for m in range(4)]
                    KCH = NF // 128  # 32
                    hb = TB // 2
                    for k in range(KCH):
                        cw_t = wpool.tile([128, N], MDT)
                        nc.sync.dma_start(cw_t[:, :], mdt(cw_d[128 * k:128 * (k + 1), :]))
                        xt_t = xpool.tile([128, TB], MDT)
                        nc.sync.dma_start(xt_t[:, :], mdt(xT_d[128 * k:128 * (k + 1), :]))
                        for m in range(4):
                            for j, (s, e) in enumerate(pieces):
                                mm(pAB[m][j][:, :], cw_t[:, 128 * m:128 * (m + 1)],
                                   xt_t[:, s:e],
                                   start=(k == 0), stop=(k == KCH - 1))
                    for m in range(4):
                        for j, (s, e) in enumerate(pieces):
                            nc.vector.tensor_copy(xrT_s[:, m, s:e], pAB[m][j][:, :])

                # =========== Phase 1B: xz_t = xrT_t.T @ k_p + cb2 ===========
                with ExitStack() as p1b:
                    sbB = p1b.enter_context(tc.tile_pool(name="p1bsb", bufs=1))
                    k_s = sbB.tile([128, 4, G4], MDT)
                    for kk in range(4):
                        nc.sync.dma_start(k_s[:, kk, :], mdt(kp_d[128 * kk:128 * (kk + 1), :]))
                    psB = p1b.enter_context(tc.tile_pool(name="psB", bufs=2, space="PSUM"))
                    evac = p1b.enter_context(tc.tile_pool(name="evac", bufs=3))
                    for t in range(HC):
                        z_ps = psB.tile([128, N], F32)  # (gate,batch)-major
                        for q in range(4):
                            mm(z_ps[32 * q:32 * (q + 1), :], ones_s[:, :],
                               cb2_s[:, N * q:N * (q + 1)], start=True, stop=False, q=q)
                        for kk in range(4):
                            for q in range(4):
                                mm(z_ps[32 * q:32 * (q + 1), :],
                                   xrT_s[:, kk, 32 * t:32 * (t + 1)],
                                   k_s[:, kk, N * q:N * (q + 1)],
                                   start=False, stop=(kk == 3), q=q)
                        # relocate to batch-major [32, 4, 512] (ACT may cross
                        # partition windows; DVE may not)
                        xz_sb = evac.tile([B, 4, N], MDT)
                        for q in range(4):
                            nc.scalar.activation(xz_sb[:, q, :],
                                                 z_ps[32 * q:32 * (q + 1), :], AF.Copy)
                        nc.sync.dma_start(mdt(xz_loc[B * t:B * (t + 1), :]),
                                          xz_sb[:, :, :].rearrange("b q n -> b (q n)"))

            # ======================== AllGather ========================
            if timing:
                # TimelineSim can't model collectives: stand in 8 local DRAM
                # copies with the same data volume as the AllGather.
                for rr in range(CORES):
                    nc.sync.dma_start(
                        xz_all[HC * B * rr:HC * B * (rr + 1), :], xz_loc[:, :])
            else:
                nc.gpsimd.collective_compute(
                    "AllGather", bass.mybir.AluOpType.bypass,
                    replica_groups=[list(range(CORES))],
                    ins=[xz_loc.opt()], outs=[xz_all.opt()],
                )

            # ================= Phase 2: the recurrence =================
            sb2 = octx.enter_context(tc.tile_pool(name="p2sb", bufs=1))
            rk_s = sb2.tile([128, 4, G4], MDT)
            wd_s = sb2.tile([128, 4, G4], MDT)
            for kk in range(4):
                nc.sync.dma_start(rk_s[:, kk, :], mdt(rkp_d[128 * kk:128 * (kk + 1), :]))
                nc.sync.dma_start(wd_s[:, kk, :], mdt(wdp_d[128 * kk:128 * (kk + 1), :]))

            hT_f = sb2.tile([128, 128], F32)
            nc.vector.memset(hT_f[:], 0.0)
            hT_s = sb2.tile([128, 128], MDT)
            nc.vector.tensor_copy(hT_s[:], hT_f[:])
            c_s = sb2.tile([B, N], F32)
            nc.vector.memset(c_s[:], 0.0)
            si_s = sb2.tile([B, N], F32)       # sigmoid(i)
            sf_s = sb2.tile([B, N], F32)       # sigmoid(f)
            so_s = sb2.tile([B, N], F32)       # sigmoid(o)
            tg_s = sb2.tile([B, N], F32)       # tanh(g)
            t1_s = sb2.tile([B, N], F32)
            t2_s = sb2.tile([B, N], F32)
            tc_s = sb2.tile([B, N], F32)
            h_s = sb2.tile([B, N], F32)
            predsT_s = sb2.tile([128, 4, P_STEPS * B], MDT)

            xzp = octx.enter_context(tc.tile_pool(name="xzp", bufs=8))
            ps2 = octx.enter_context(tc.tile_pool(name="ps2", bufs=3, space="PSUM"))
            trp = octx.enter_context(tc.tile_pool(name="trp", bufs=2, space="PSUM"))

            def step(t, enc):
                """One LSTM step, batch-major [32, 2048]: all elementwise ops
                stay on partitions 0:32; gates live in the free dimension."""
                z_ps = ps2.tile([128, N], F32, name="z_ps")  # (gate,batch)-major
                if enc:
                    xz_t = xzp.tile([B, 4, N], MDT, name="xz_t")
                    nc.sync.dma_start(xz_t[:, :, :].rearrange("b q n -> b (q n)"),
                                      mdt(xz_all[B * t:B * (t + 1), :]))
                    w_s = rk_s
                else:
                    w_s = wd_s
                if enc or t == 0:
                    lhs = lambda kk: hT_s[:, 32 * kk:32 * (kk + 1)]
                else:
                    lhs = lambda kk: predsT_s[:, kk, B * (t - 1):B * t]
                tr = trp.tile([128, 128], F32, name="tr")
                # Pipeline the feature dim in halves: emit all matmuls first
                # (every half contracts over the FULL previous h), then run
                # each half's activations/cell-update/transposes — half A's
                # ACT/DVE work overlaps half B's matmuls on the PE.
                for h0 in (0, N // 2):
                    h1 = h0 + N // 2
                    # inject xz (enc) / bias (dec) via identity/ones matmuls,
                    # one per column group (concurrent on the PE array)
                    for q in range(4):
                        if enc:
                            mm(z_ps[32 * q:32 * (q + 1), h0:h1], identr[:, :],
                               xz_t[:, q, h0:h1], start=True, stop=False, q=q)
                        else:
                            mm(z_ps[32 * q:32 * (q + 1), h0:h1], ones_s[:, :],
                               bdec_s[:, N * q + h0:N * q + h1],
                               start=True, stop=False, q=q)
                    for kk in range(4):
                        for q in range(4):
                            mm(z_ps[32 * q:32 * (q + 1), h0:h1], lhs(kk),
                               w_s[:, kk, N * q + h0:N * q + h1],
                               start=False, stop=(kk == 3), q=q)
                for h0 in (0, N // 2):
                    h1 = h0 + N // 2
                    # activations, relocating each gate to partitions 0:32
                    # (ACT cross-window reads are legal; DVE stays aligned).
                    # slots: [i | f | o | g]
                    sl = slice(h0, h1)
                    nc.scalar.activation(sf_s[:, sl], z_ps[32:64, sl], AF.Sigmoid)
                    nc.scalar.activation(tg_s[:, sl], z_ps[96:128, sl], AF.Tanh)
                    nc.scalar.activation(si_s[:, sl], z_ps[0:32, sl], AF.Sigmoid)
                    nc.scalar.activation(so_s[:, sl], z_ps[64:96, sl], AF.Sigmoid)
                    # c = f*c + i*g ; h = o * tanh(c)
                    nc.vector.tensor_mul(t2_s[:, sl], sf_s[:, sl], c_s[:, sl])
                    nc.vector.tensor_mul(t1_s[:, sl], si_s[:, sl], tg_s[:, sl])
                    nc.vector.tensor_add(c_s[:, sl], t1_s[:, sl], t2_s[:, sl])
                    nc.scalar.activation(tc_s[:, sl], c_s[:, sl], AF.Tanh)
                    nc.vector.tensor_mul(h_s[:, sl], so_s[:, sl], tc_s[:, sl])
                    # transpose this half's h chunks to feature-major
                    for kk in (h0 // 128, h0 // 128 + 1):
                        nc.tensor.transpose(tr[:, 32 * kk:32 * (kk + 1)],
                                            h_s[:, 128 * kk:128 * (kk + 1)],
                                            ident[:, :])
                        dst = (hT_s[:, 32 * kk:32 * (kk + 1)] if enc
                               else predsT_s[:, kk, B * t:B * (t + 1)])
                        nc.vector.tensor_copy(dst, tr[:, 32 * kk:32 * (kk + 1)])

            for t in range(H):
                step(t, enc=True)
            for t in range(P_STEPS):
                step(t, enc=False)

            # ================= Phase 3: dense head =================
            dw_s = sb2.tile([128, 4, N], MDT)
            for kk in range(4):
                nc.sync.dma_start(dw_s[:, kk, :], mdt(dw_d[128 * kk:128 * (kk + 1), :]))
            db_s = sb2.tile([128, N], F32)
            nc.sync.dma_start(db_s[:, :], dbb_d[:, :])
            ps3 = octx.enter_context(tc.tile_pool(name="ps3", bufs=2, space="PSUM"))
            op3 = octx.enter_context(tc.tile_pool(name="op3", bufs=2))
            MT = (P_STEPS * B) // 128  # 6
            out_r = out_d[:, :, :].rearrange("b t n -> t b n")
            for m in range(MT):
                dps = ps3.tile([128, N], F32, name="dps")
                for kk in range(4):
                    mm(dps[:, :], predsT_s[:, kk, 128 * m:128 * (m + 1)],
                       dw_s[:, kk, :], start=(kk == 0), stop=(kk == 3))
                o_sb = op3.tile([128, N], F32, name="o_sb")
                nc.vector.tensor_add(o_sb[:, :], dps[:, :], db_s[:, :])
                for j in range(4):
                    nc.sync.dma_start(out_r[4 * m + j, :, :], o_sb[B * j:B * (j + 1), :])
    return nc


def _host_prep(x, conv_w, conv_b, lstm_k, lstm_rk, lstm_b, dense_w, dense_b):
    x = np.asarray(x, dtype=np.float32)
    conv_w = np.asarray(conv_w, dtype=np.float32)
    conv_b = np.asarray(conv_b, dtype=np.float32)
    lstm_k = np.asarray(lstm_k, dtype=np.float32)
    lstm_rk = np.asarray(lstm_rk, dtype=np.float32)
    lstm_b = np.asarray(lstm_b, dtype=np.float32)
    dense_w = np.asarray(dense_w, dtype=np.float32)
    dense_b = np.asarray(dense_b, dtype=np.float32)

    # gate slot order [i | f | o | g] (keras order in weights is i,f,g,o)
    perm = np.r_[0:N, N:2 * N, 3 * N:4 * N, 2 * N:3 * N]
    k_p = np.ascontiguousarray(lstm_k[:, perm])
    rk_p = np.ascontiguousarray(lstm_rk[:, perm])
    wdec_p = np.ascontiguousarray((lstm_k + lstm_rk)[:, perm])
    cb2 = (conv_b @ lstm_k + lstm_b)[perm].reshape(4, N)
    bdec = lstm_b[perm].reshape(4, N)
    db_bc = np.ascontiguousarray(np.broadcast_to(dense_b, (128, N)))

    shared = {
        "conv_w": conv_w, "k_p": k_p, "rk_p": rk_p, "wdec_p": wdec_p,
        "cb2": np.ascontiguousarray(cb2), "bdec": np.ascontiguousarray(bdec),
        "dense_w": dense_w, "dense_b_bc": db_bc,
    }
    in_maps = []
    for c in range(CORES):
        xs = x[:, HC * c:HC * (c + 1)].reshape(B, HC, NF)
        xT = np.ascontiguousarray(xs.transpose(2, 1, 0).reshape(NF, TB))
        in_maps.append({"xT": xT, **shared})
    return in_maps


class _Runner:
    """Builds the SPMD PJRT executable once; reusable across calls.

    Mirrors concourse.bass2jax.run_bass_via_pjrt's multi-core branch but
    keeps the jitted callable so repeat executions don't recompile.
    """

    def __init__(self, nc):
        import jax
        from jax.sharding import Mesh, PartitionSpec, NamedSharding
        from jax.experimental.shard_map import shard_map
        from concourse import bass2jax, mybir

        bass2jax.install_neuronx_cc_hook()
        if not nc.is_finalized():
            nc.finalize()
        self.nc = nc
        partition_name = (nc.partition_id_tensor.name
                          if nc.partition_id_tensor else None)
        in_names, out_names, out_avals = [], [], []
        for alloc in nc.m.functions[0].allocations:
            if not isinstance(alloc, mybir.MemoryLocationSet):
                continue
            name = alloc.memorylocations[0].name
            if alloc.kind == "ExternalInput":
                if name != partition_name:
                    in_names.append(name)
            elif alloc.kind == "ExternalOutput":
                shape = tuple(alloc.tensor_shape)
                dtype = mybir.dt.np(alloc.dtype)
                out_names.append(name)
                out_avals.append(jax.core.ShapedArray(shape, dtype))
        self.n_params = len(in_names)
        self.in_names = list(in_names)
        self.out_names = out_names
        self.out_avals = out_avals
        all_names = in_names + out_names
        if partition_name is not None:
            all_names.append(partition_name)

        def _body(*args):
            operands = list(args)
            if partition_name is not None:
                operands.append(bass2jax.partition_id_tensor())
            outs = bass2jax._bass_exec_p.bind(
                *operands,
                out_avals=tuple(out_avals),
                in_names=tuple(all_names),
                out_names=tuple(out_names),
                lowering_input_output_aliases=(),
                sim_require_finite=True,
                sim_require_nnan=True,
                nc=nc,
            )
            return tuple(outs)

        devices = jax.devices()[:CORES]
        self.mesh = Mesh(np.asarray(devices), ("core",))
        self.sharding = NamedSharding(self.mesh, PartitionSpec("core"))
        n_io = self.n_params + len(out_names)
        self.fn = jax.jit(
            shard_map(_body, mesh=self.mesh,
                      in_specs=(PartitionSpec("core"),) * n_io,
                      out_specs=(PartitionSpec("core"),) * len(out_names),
                      check_rep=False),
            keep_unused=True)
        self._jax = jax
        self._zeros = None

    def put_inputs(self, in_maps):
        jax = self._jax
        concat = [np.concatenate([np.asarray(m[n]) for m in in_maps], axis=0)
                  for n in self.in_names]
        return [jax.device_put(a, self.sharding) for a in concat]

    def fresh_zeros(self):
        jax = self._jax
        if self._zeros is None:
            self._zeros = [jax.device_put(
                np.zeros((CORES * av.shape[0], *av.shape[1:]), av.dtype),
                self.sharding) for av in self.out_avals]
        return self._zeros

    def run(self, dev_in):
        outs = self.fn(*dev_in, *self.fresh_zeros())
        return [np.asarray(o) for o in outs]


def _get_runner():
    if "runner" not in _CACHE:
        _CACHE["runner"] = _Runner(_build_nc())
    return _CACHE["runner"]


def kernel(x, conv_w, conv_b, lstm_k, lstm_rk, lstm_b, dense_w, dense_b, P):
    in_maps = _host_prep(x, conv_w, conv_b, lstm_k, lstm_rk,
                         lstm_b, dense_w, dense_b)
    last_err = None
    for attempt in range(3):
        try:
            r = _get_runner()
            outs = r.run(r.put_inputs(in_maps))
            full = outs[r.out_names.index("out")]
            return (np.asarray(full).reshape(CORES, B, P_STEPS, N)[0]
                    .astype(np.float32))
        except Exception as e:  # transient NRT wedges recover on retry
            last_err = e
            _CACHE.clear()
            import time as _time
            _time.sleep(5)
    raise last_err
